# revision 5
# baseline (speedup 1.0000x reference)
"""Trainium2 Bass kernel for nn_DecompGen (conditional generator + rank-decomposed
outer-product head).

Sharding: pure data parallelism — batch B=256 is split 32-per-core across 8
NeuronCores; the small parameter set is replicated.  Training-mode BatchNorm
needs full-batch statistics, so the kernel does two tiny AllReduces (one for
the stage-A convs' stats, one for stage-B).  The first BN (on the input
linear) is handled by replicating that stage's trivially small matmul on
every core, which removes a third AllReduce.

Layout conventions (per core, channels on SBUF partitions):
  - conv activations are stored "t-major": tile column = t*32 + b
  - convT is computed as K shifted matmuls accumulating into one PSUM bank;
    each weight-tap k contributes to output window t in [k, k+L_in); windows
    are emitted as (covered + fresh) split matmuls so per-element PSUM
    has_written semantics stay uniform per instruction.
  - the rank-weighted outer-product head runs per-sample matmuls
    out[(ci,hi), wi] = sum_r (coef*c)[r,ci]*h[r,hi] . w[r,wi]
"""

import os
import threading

import numpy as np

import concourse.bacc as bacc
import concourse.mybir as mybir
import concourse.tile as tile
from concourse.bass_utils import run_bass_kernel_spmd

F32 = mybir.dt.float32
F32R = mybir.dt.float32r
BF16 = mybir.dt.bfloat16

N_CORES = 8
B, NOISE, NCLASS, RANK = 256, 100, 10, 512
R4, R2 = RANK // 4, RANK // 2  # 128, 256
BC = B // N_CORES  # 32 samples per core
EPS = 1e-5

# precision config: "f32" (exact), "f32r" (fp32 storage, fast PE mode on the
# big convs), "bf16" (bf16 weights/activations on the conv + einsum path)
PREC = os.environ.get("KERNEL_PREC", "f32r")

_lock = threading.Lock()
_cache: dict = {}


def _cdt():
    """storage dtype of conv weights/activations on the B/C conv path"""
    return BF16 if PREC == "bf16" else F32


def _mmbc(ap):
    """matmul operand view for stage B/C convs"""
    if PREC == "f32r":
        return ap.bitcast(F32R)
    return ap


# --------------------------------------------------------------------------
# device kernel
# --------------------------------------------------------------------------

def _emit(nc, tc, I, out_ap):
    ADT = _cdt()          # conv activations (h1/h2, latent)
    WDT = _cdt()          # conv weights (host already cast)
    GDT = BF16   # einsum G / w3 path (always bf16: feeds fp32-accum matmuls)
    Alu = mybir.AluOpType
    Act = mybir.ActivationFunctionType

    sb = tc.alloc_tile_pool(name="sb", bufs=1)
    # all PSUM tiles are <= one bank; share 8 rotating bank slots
    psp = tc.alloc_tile_pool(name="ps", bufs=8, space="PSUM")
    dram = tc.alloc_tile_pool(name="dram", bufs=1, space="DRAM")

    def load(name, shape, dtype=F32):
        t = sb.tile(list(shape), dtype, name=f"sb_{name}")
        nc.sync.dma_start(t[:], I[name])
        return t

    # ---- input loads (small/latency-critical first) ----
    noise_my = load("noise_my", (NOISE, BC))
    onehot = load("onehot", (NCLASS, BC))
    lin_wt = load("lin_wt", (NOISE, 128))
    emb_w = load("emb_w", (NCLASS, 128))
    noise_t = load("noise_t", (NOISE, B))
    bn0_g = load("bn0_g", (128, 1))
    bn0_b = load("bn0_b", (128, 1))
    gA = load("gA", (128, 6))
    beA = load("beA", (128, 6))
    coef_p = load("coef_p", (128, 4))
    gB = load("gB", (128, 4))
    beB = load("beB", (128, 4))
    b3h = load("b3h", (128, 4))
    b3w = load("b3w", (128, 4))
    wA_c = load("wA_c", (128, 2 * 3 * 4 * 128), WDT)
    wA_h = load("wA_h", (128, 2 * 16 * 128), WDT)
    wA_w = load("wA_w", (128, 2 * 16 * 128), WDT)
    wB_h = load("wB_h", (128, 16 * 2 * 128), WDT)
    wB_w = load("wB_w", (128, 16 * 2 * 128), WDT)
    wC_h = load("wC_h", (128, 2 * 2 * 4 * 128), WDT)
    wC_w = load("wC_w", (128, 2 * 2 * 4 * 128), WDT)

    eps_c = sb.tile([128, 1], F32, name="eps_c")
    nc.vector.memset(eps_c[:], EPS)

    # ======================= stage 0 (replicated) =========================
    # z_pre^T = lin_w @ noise^T ; full batch for the BN stats, own shard for
    # the actual activations.
    ps_zf = psp.tile([128, B], F32, name="ps_zf", tag="ps")
    nc.tensor.matmul(ps_zf[:], lin_wt[:], noise_t[:], start=True, stop=True)
    ps_zmy = psp.tile([128, BC], F32, name="ps_zmy", tag="ps")
    nc.tensor.matmul(ps_zmy[:], lin_wt[:], noise_my[:], start=True, stop=True)
    ps_lab = psp.tile([128, BC], F32, name="ps_lab", tag="ps")
    nc.tensor.matmul(ps_lab[:], emb_w[:], onehot[:], start=True, stop=True)

    st0 = sb.tile([128, 2], F32, name="st0")
    scr0 = sb.tile([128, 512], F32, name="scr0", tag="scr", bufs=2)
    nc.vector.tensor_scalar(scr0[:, 0:B], ps_zf[:], 1.0, None, Alu.mult, Alu.add,
                            accum_out=st0[:, 0:1])
    nc.scalar.activation(scr0[:, 0:B], ps_zf[:], Act.Square, accum_out=st0[:, 1:2])

    # per-channel BN0 coefficients (count = B; lin_b cancels under BN)
    m0 = sb.tile([128, 2], F32, name="m0")
    nc.vector.tensor_scalar(m0[:], st0[:], 1.0 / B, None, Alu.mult)
    msq0 = sb.tile([128, 1], F32, name="msq0")
    nc.scalar.activation(msq0[:], m0[:, 0:1], Act.Square)
    var0 = sb.tile([128, 1], F32, name="var0")
    nc.vector.tensor_tensor(var0[:], m0[:, 1:2], msq0[:], Alu.subtract)
    std0 = sb.tile([128, 1], F32, name="std0")
    nc.scalar.activation(std0[:], var0[:], Act.Sqrt, bias=eps_c[:])
    rstd0 = sb.tile([128, 1], F32, name="rstd0")
    nc.vector.reciprocal(rstd0[:], std0[:])
    A0 = sb.tile([128, 1], F32, name="A0")
    nc.vector.tensor_tensor(A0[:], rstd0[:], bn0_g[:], Alu.mult)
    t0_ = sb.tile([128, 1], F32, name="t0_")
    nc.vector.tensor_tensor(t0_[:], m0[:, 0:1], A0[:], Alu.mult)
    B0 = sb.tile([128, 1], F32, name="B0")
    nc.vector.tensor_tensor(B0[:], bn0_b[:], t0_[:], Alu.subtract)

    # latent = [lrelu(bn0(z)) ; emb[label]]  (2 chunks of 128 channels)
    lat0 = sb.tile([128, BC], ADT, name="lat0")
    nc.scalar.activation(lat0[:], ps_zmy[:], Act.Identity, bias=B0[:], scale=A0[:])
    nc.vector.scalar_tensor_tensor(lat0[:], lat0[:], 0.01, lat0[:], Alu.mult, Alu.max)
    lat1 = sb.tile([128, BC], ADT, name="lat1")
    nc.vector.tensor_copy(lat1[:], ps_lab[:])
    lat = [lat0, lat1]

    # ======================= stage A convs (L_in=1) =======================
    # x[co, t, b] = sum_ci w[ci, co, t] * latent[ci, b];  psum cols t*32+b
    stA = sb.tile([128, 12], F32, name="stA")

    def stageA_conv(wtile, KA, G, s1cols, name):
        """returns list of evacuated pre-BN sbuf tiles (one per co-group)"""
        outs = []
        for g in range(G):
            ps = psp.tile([128, KA * BC], F32, name=f"psA_{name}{g}", tag="ps")
            n_mm = KA * 2
            i = 0
            for k in range(KA):
                for chunk in range(2):
                    idx = ((chunk * KA + k) * G + g) * 128
                    nc.tensor.matmul(ps[:, k * BC:(k + 1) * BC],
                                     wtile[:, idx:idx + 128], lat[chunk][:],
                                     start=(i == 0), stop=(i == n_mm - 1))
                    i += 1
            x = sb.tile([128, KA * BC], F32, name=f"xA_{name}{g}")
            col = s1cols + g
            nc.vector.tensor_scalar(x[:], ps[:], 1.0, None, Alu.mult, Alu.add,
                                    accum_out=stA[:, col:col + 1])
            scr = sb.tile([128, 512], F32, name=f"scrA_{name}{g}", tag="scr", bufs=2)
            nc.scalar.activation(scr[:, 0:KA * BC], ps[:], Act.Square,
                                 accum_out=stA[:, 6 + col:7 + col])
            outs.append(x)
        return outs

    xA_h = stageA_conv(wA_h, 16, 1, 0, "h")[0]
    xA_w = stageA_conv(wA_w, 16, 1, 1, "w")[0]
    xA_c = stageA_conv(wA_c, 3, 4, 2, "c")

    # ---- AllReduce #1: stage-A BN stats ----
    arA_in = dram.tile([128, 12], F32, name="arA_in")
    arA_out = dram.tile([128, 12], F32, name="arA_out")
    nc.gpsimd.dma_start(arA_in[:], stA[:])
    nc.gpsimd.collective_compute("AllReduce", Alu.add,
                                 replica_groups=[list(range(N_CORES))],
                                 ins=[arA_in.opt()], outs=[arA_out.opt()])
    stAg = sb.tile([128, 12], F32, name="stAg")
    nc.gpsimd.dma_start(stAg[:], arA_out[:])

    # ---- BN coefficients for stage A (counts: h/w 256*16, c 256*3) ----
    def bn_coeffs(stg, ncols, counts, g_t, be_t, name):
        mean = sb.tile([128, ncols], F32, name=f"mean{name}")
        ex2 = sb.tile([128, ncols], F32, name=f"ex2{name}")
        for sl, cnt in counts:
            nc.vector.tensor_scalar(mean[:, sl], stg[:, sl], 1.0 / cnt, None, Alu.mult)
            s2 = slice(sl.start + ncols, sl.stop + ncols)
            nc.vector.tensor_scalar(ex2[:, sl], stg[:, s2], 1.0 / cnt, None, Alu.mult)
        msq = sb.tile([128, ncols], F32, name=f"msq{name}")
        nc.scalar.activation(msq[:], mean[:], Act.Square)
        var = sb.tile([128, ncols], F32, name=f"var{name}")
        nc.vector.tensor_tensor(var[:], ex2[:], msq[:], Alu.subtract)
        std = sb.tile([128, ncols], F32, name=f"std{name}")
        nc.scalar.activation(std[:], var[:], Act.Sqrt, bias=eps_c[:])
        rstd = sb.tile([128, ncols], F32, name=f"rstd{name}")
        nc.vector.reciprocal(rstd[:], std[:])
        Atl = sb.tile([128, ncols], F32, name=f"A{name}")
        nc.vector.tensor_tensor(Atl[:], rstd[:], g_t[:], Alu.mult)
        tmp = sb.tile([128, ncols], F32, name=f"tmp{name}")
        nc.vector.tensor_tensor(tmp[:], mean[:], Atl[:], Alu.mult)
        Btl = sb.tile([128, ncols], F32, name=f"B{name}")
        nc.vector.tensor_tensor(Btl[:], be_t[:], tmp[:], Alu.subtract)
        return Atl, Btl

    A_A, B_A = bn_coeffs(stAg, 6, [(slice(0, 2), B * 16), (slice(2, 6), B * 3)],
                         gA, beA, "A")
    # fold the rank weights into the c-path coefficients (coef >= 0 commutes
    # with leaky-relu)
    nc.vector.tensor_tensor(A_A[:, 2:6], A_A[:, 2:6], coef_p[:], Alu.mult)
    nc.vector.tensor_tensor(B_A[:, 2:6], B_A[:, 2:6], coef_p[:], Alu.mult)

    def bn_apply(dst, src, Atl, Btl, col, slope):
        nc.scalar.activation(dst, src, Act.Identity,
                             bias=Btl[:, col:col + 1], scale=Atl[:, col:col + 1])
        nc.vector.scalar_tensor_tensor(dst, dst, slope, dst, Alu.mult, Alu.max)

    if ADT == F32:
        h1, w1 = xA_h, xA_w
        bn_apply(h1[:], h1[:], A_A, B_A, 0, 0.2)
        bn_apply(w1[:], w1[:], A_A, B_A, 1, 0.2)
    else:
        h1 = sb.tile([128, 16 * BC], ADT, name="h1")
        bn_apply(h1[:], xA_h[:], A_A, B_A, 0, 0.2)
        w1 = sb.tile([128, 16 * BC], ADT, name="w1")
        bn_apply(w1[:], xA_w[:], A_A, B_A, 1, 0.2)
    # c~ = coef * lrelu(bn(c_pre)): written b-major (cols b*3+ci) for the head
    cT = []
    for g in range(4):
        ct = sb.tile([128, 3 * BC], F32, name=f"cT{g}")
        src = xA_c[g][:].rearrange("p (c b) -> p c b", b=BC)
        dstv = ct[:].rearrange("p (b c) -> p c b", c=3)
        bn_apply(dstv, src, A_A, B_A, 2 + g, 0.2)
        cT.append(ct)

    # ======================= stage B convs (16 -> 31) =====================
    # y[co, t, b] += sum_ci w2[ci, co, k] * h1[ci, t-k, b]
    stB = sb.tile([128, 16], F32, name="stB")

    def convT(wtile, src_chunks, LI, LO, KK, G, psname, n_bh=2, widx=None):
        """shifted-window convT: returns psum tiles [(g, bh) -> [128, LO*16]]"""
        pss = {}
        n_ck = len(src_chunks)
        for g in range(G):
            for bh in range(n_bh):
                ps = psp.tile([128, LO * 16], F32, name=f"ps{psname}_{g}_{bh}",
                              tag="ps")
                n_mm = KK * n_ck + (KK - 1) * n_ck  # full + split pairs
                i = 0
                for k in range(KK):
                    for ck in range(n_ck):
                        lw = wtile[:, widx(ck, k, g):widx(ck, k, g) + 128]
                        src = src_chunks[ck]
                        if k == 0:
                            rhs = src[:].rearrange("p (t b) -> p t b", b=BC)[
                                :, 0:LI, bh * 16:(bh + 1) * 16]
                            nc.tensor.matmul(ps[:, 0:LI * 16], lw, rhs,
                                             start=(i == 0), stop=False)
                            i += 1
                        else:
                            rhs = src[:].rearrange("p (t b) -> p t b", b=BC)[
                                :, 0:LI - 1, bh * 16:(bh + 1) * 16]
                            nc.tensor.matmul(
                                ps[:, k * 16:(k + LI - 1) * 16], lw, rhs,
                                start=False, stop=False)
                            i += 1
                            rhs2 = src[:, (LI - 1) * BC + bh * 16:
                                       (LI - 1) * BC + bh * 16 + 16]
                            last = (k == KK - 1 and ck == n_ck - 1)
                            nc.tensor.matmul(
                                ps[:, (k + LI - 1) * 16:(k + LI) * 16], lw, rhs2,
                                start=False, stop=last)
                            i += 1
                pss[(g, bh)] = ps
        return pss

    psB_h = convT(_W(wB_h), [_A(h1)], 16, 31, 16, 2, "Bh",
                  widx=lambda ck, k, g: (k * 2 + g) * 128)
    psB_w = convT(_W(wB_w), [_A(w1)], 16, 31, 16, 2, "Bw",
                  widx=lambda ck, k, g: (k * 2 + g) * 128)

    # evacuate + stats; xB tiles are [128, 31*BC] t-major over full b
    def evacB(pss, G, base, name):
        outs = []
        for g in range(G):
            x = sb.tile([128, 31 * BC], F32, name=f"xB_{name}{g}")
            for bh in range(2):
                ps = pss[(g, bh)]
                col = base + g * 2 + bh
                dstv = x[:].rearrange("p (t b) -> p t b", b=BC)[:, :, bh * 16:(bh + 1) * 16]
                nc.vector.tensor_scalar(dstv, ps[:], 1.0, None, Alu.mult, Alu.add,
                                        accum_out=stB[:, col:col + 1])
                scr = sb.tile([128, 512], F32, name=f"scrB_{name}{g}{bh}",
                              tag="scr", bufs=2)
                nc.scalar.activation(scr[:, 0:31 * 16], ps[:], Act.Square,
                                     accum_out=stB[:, 8 + col:9 + col])
            outs.append(x)
        return outs

    xB_h = evacB(psB_h, 2, 0, "h")
    xB_w = evacB(psB_w, 2, 4, "w")

    # ---- AllReduce #2: stage-B BN stats ----
    arB_in = dram.tile([128, 16], F32, name="arB_in")
    arB_out = dram.tile([128, 16], F32, name="arB_out")
    nc.gpsimd.dma_start(arB_in[:], stB[:])
    nc.gpsimd.collective_compute("AllReduce", Alu.add,
                                 replica_groups=[list(range(N_CORES))],
                                 ins=[arB_in.opt()], outs=[arB_out.opt()])
    stBg = sb.tile([128, 16], F32, name="stBg")
    nc.gpsimd.dma_start(stBg[:], arB_out[:])

    # combine the two batch-half partial sums, then BN coefficients
    stBs = sb.tile([128, 8], F32, name="stBs")
    v = stBg[:].rearrange("p (c two) -> p c two", two=2)
    nc.vector.tensor_tensor(stBs[:], v[:, :, 0:1].squeeze(2), v[:, :, 1:2].squeeze(2),
                            Alu.add)
    A_B, B_B = bn_coeffs(stBs, 4, [(slice(0, 4), B * 31)], gB, beB, "Bst")

    h2, w2 = [], []
    for g in range(2):
        if ADT == F32:
            t = xB_h[g]
            bn_apply(t[:], t[:], A_B, B_B, g, 0.2)
        else:
            t = sb.tile([128, 31 * BC], ADT, name=f"h2_{g}")
            bn_apply(t[:], xB_h[g][:], A_B, B_B, g, 0.2)
        h2.append(t)
        if ADT == F32:
            t = xB_w[g]
            bn_apply(t[:], t[:], A_B, B_B, 2 + g, 0.2)
        else:
            t = sb.tile([128, 31 * BC], ADT, name=f"w2_{g}")
            bn_apply(t[:], xB_w[g][:], A_B, B_B, 2 + g, 0.2)
        w2.append(t)

    # ======================= stage C convs (31 -> 32) =====================
    def stageC(wtile, src_chunks, b3t, name, out_dt):
        pss = convT(_W(wtile), [_A(s) for s in src_chunks], 31, 32, 2, 4, name,
                    widx=lambda ck, k, g: ((ck * 2 + k) * 4 + g) * 128)
        outs = []
        for g in range(4):
            h = sb.tile([128, BC * 32], out_dt, name=f"{name}3_{g}")
            for bh in range(2):
                ps = pss[(g, bh)]
                # psum cols t*16+j -> sbuf cols (bh*16+j)*32 + t
                inv = ps[:].rearrange("p (t j) -> p j t", j=16)
                dstv = h[:].rearrange("p (b t) -> p b t", t=32)[
                    :, bh * 16:(bh + 1) * 16, :]
                nc.scalar.activation(dstv, inv, Act.Tanh, bias=b3t[:, g:g + 1])
            outs.append(h)
        return outs

    h3 = stageC(wC_h, h2, b3h, "h", F32)
    GDTt = GDT
    w3 = stageC(wC_w, w2, b3w, "w", GDTt)

    # ======================= rank-weighted outer-product head =============
    # G[q][r, b*96 + ci*32 + hi] = c~[q][r, b*3+ci] * h3[q][r, b*32+hi]
    Gt = []
    for q in range(4):
        g = sb.tile([128, BC * 96], GDTt, name=f"G{q}")
        cv = cT[q][:].rearrange("p (b c) -> p b c", c=3).unsqueeze(3) \
            .broadcast_to([128, BC, 3, 32])
        hv = h3[q][:].rearrange("p (b h) -> p b h", h=32).unsqueeze(2) \
            .broadcast_to([128, BC, 3, 32])
        gv = g[:].rearrange("p (b c h) -> p b c h", c=3, h=32)
        nc.vector.tensor_tensor(gv, cv, hv, Alu.mult)
        Gt.append(g)

    outsb = sb.tile([96, BC * 32], F32, name="outsb")
    for grp in range(BC // 4):
        po = psp.tile([96, 4 * 32], F32, name=f"po{grp}", tag="ps")
        for j in range(4):
            bb = grp * 4 + j
            for q in range(4):
                nc.tensor.matmul(po[:, j * 32:(j + 1) * 32],
                                 Gt[q][:, bb * 96:(bb + 1) * 96],
                                 w3[q][:, bb * 32:(bb + 1) * 32],
                                 start=(j == 0 and q == 0),
                                 stop=(j == 3 and q == 3))
        nc.vector.tensor_copy(outsb[:, grp * 128:(grp + 1) * 128], po[:])

    # out[b, (ci,hi), wi] <- outsb[(ci,hi), b*32+wi]
    out_v = out_ap.rearrange("b c h w -> (c h) b w")
    nc.sync.dma_start(out_v, outsb[:].rearrange("p (b w) -> p b w", w=32))

    sb.release()
    psp.release()
    dram.release()


def _W(wtile):
    return _WView(wtile)


class _WView:
    """weight tile wrapper applying the f32r bitcast at slice time"""

    def __init__(self, t):
        self.t = t

    def __getitem__(self, key):
        return _mmbc(self.t[key])


class _AView:
    """activation tile wrapper applying the f32r bitcast at slice time"""

    def __init__(self, t):
        self.t = t

    def __getitem__(self, key):
        return _mmbc(self.t[key])


def _A(t):
    return _AView(t)


# --------------------------------------------------------------------------
# host side
# --------------------------------------------------------------------------

def _build_module():
    nc = bacc.Bacc("TRN2", target_bir_lowering=False, debug=False,
                   num_devices=N_CORES)
    WDT = _cdt()
    specs = {
        "noise_my": ((NOISE, BC), F32), "onehot": ((NCLASS, BC), F32),
        "lin_wt": ((NOISE, 128), F32), "emb_w": ((NCLASS, 128), F32),
        "noise_t": ((NOISE, B), F32),
        "bn0_g": ((128, 1), F32), "bn0_b": ((128, 1), F32),
        "gA": ((128, 6), F32), "beA": ((128, 6), F32),
        "coef_p": ((128, 4), F32), "gB": ((128, 4), F32), "beB": ((128, 4), F32),
        "b3h": ((128, 4), F32), "b3w": ((128, 4), F32),
        "wA_c": ((128, 3072), WDT), "wA_h": ((128, 4096), WDT),
        "wA_w": ((128, 4096), WDT),
        "wB_h": ((128, 4096), WDT), "wB_w": ((128, 4096), WDT),
        "wC_h": ((128, 2048), WDT), "wC_w": ((128, 2048), WDT),
    }
    I = {}
    for name, (shape, dt) in specs.items():
        I[name] = nc.dram_tensor(name, list(shape), dt, kind="ExternalInput").ap()
    out = nc.dram_tensor("out", [BC, 3, 32, 32], F32, kind="ExternalOutput")
    with tile.TileContext(nc) as tc:
        _emit(nc, tc, I, out.ap())
    nc.compile()
    return nc


def _np(x):
    return np.ascontiguousarray(np.asarray(x, dtype=np.float32))


def _pack_inputs(inputs):
    """host-side layout packing -> (replicated dict, per-core dicts)"""
    wnp = np.dtype(mybir.dt.np(_cdt()))
    noise = _np(inputs["noise"])
    label = np.asarray(inputs["label"]).astype(np.int64)

    c_w1 = _np(inputs["c_w1"])   # (256, 512, 3)
    h_w1 = _np(inputs["h_w1"])   # (256, 128, 16)
    w_w1 = _np(inputs["w_w1"])
    h_w2 = _np(inputs["h_w2"])   # (128, 256, 16)
    w_w2 = _np(inputs["w_w2"])
    h_w3 = _np(inputs["h_w3"])   # (256, 512, 2)
    w_w3 = _np(inputs["w_w3"])

    def packA_c(w):   # -> [ci_in, (chunk, k, g, co_in)]
        return np.ascontiguousarray(
            w.reshape(2, 128, 4, 128, 3).transpose(1, 0, 4, 2, 3).reshape(128, -1))

    def packA_h(w):   # (256,128,16) -> [ci_in, (chunk, k, co)]
        return np.ascontiguousarray(
            w.reshape(2, 128, 128, 16).transpose(1, 0, 3, 2).reshape(128, -1))

    def packB(w):     # (128,256,16) -> [ci, (k, g, co_in)]
        return np.ascontiguousarray(
            w.reshape(128, 2, 128, 16).transpose(0, 3, 1, 2).reshape(128, -1))

    def packC(w):     # (256,512,2) -> [ci_in, (chunk, k, g, co_in)]
        return np.ascontiguousarray(
            w.reshape(2, 128, 4, 128, 2).transpose(1, 0, 4, 2, 3).reshape(128, -1))

    def col128(*arrs):
        return np.ascontiguousarray(
            np.concatenate([a.reshape(-1, 128).T for a in arrs], axis=1))

    rep = {
        "lin_wt": _np(inputs["lin_w"]).T.copy(),
        "emb_w": _np(inputs["emb"]),
        "noise_t": noise.T.copy(),
        "bn0_g": _np(inputs["bn0_g"]).reshape(128, 1),
        "bn0_b": _np(inputs["bn0_b"]).reshape(128, 1),
        "gA": col128(_np(inputs["h_g1"]), _np(inputs["w_g1"]), _np(inputs["c_g1"])),
        "beA": col128(_np(inputs["h_be1"]), _np(inputs["w_be1"]), _np(inputs["c_be1"])),
        "coef_p": col128(_np(inputs["coef"])),
        "gB": col128(_np(inputs["h_g2"]), _np(inputs["w_g2"])),
        "beB": col128(_np(inputs["h_be2"]), _np(inputs["w_be2"])),
        "b3h": col128(_np(inputs["h_b3"])),
        "b3w": col128(_np(inputs["w_b3"])),
        "wA_c": packA_c(c_w1).astype(wnp),
        "wA_h": packA_h(h_w1).astype(wnp),
        "wA_w": packA_h(w_w1).astype(wnp),
        "wB_h": packB(h_w2).astype(wnp),
        "wB_w": packB(w_w2).astype(wnp),
        "wC_h": packC(h_w3).astype(wnp),
        "wC_w": packC(w_w3).astype(wnp),
    }

    noise_t = rep["noise_t"]
    per_core = []
    for c in range(N_CORES):
        sl = slice(c * BC, (c + 1) * BC)
        oh = (label[sl][None, :] == np.arange(NCLASS)[:, None]).astype(np.float32)
        per_core.append({
            "noise_my": np.ascontiguousarray(noise_t[:, sl]),
            "onehot": np.ascontiguousarray(oh),
            **rep,
        })
    return per_core


def kernel(**inputs) -> np.ndarray:
    with _lock:
        nc = _cache.get(PREC)
        if nc is None:
            nc = _build_module()
            _cache[PREC] = nc
    in_maps = _pack_inputs(inputs)
    res = run_bass_kernel_spmd(nc, in_maps, core_ids=list(range(N_CORES)))
    return np.concatenate([r["out"] for r in res.results], axis=0)


# revision 9
# speedup vs baseline: 1.8159x; 1.8159x over previous
"""Trainium2 Bass kernel for nn_DecompGen (conditional generator + rank-decomposed
outer-product head).

Sharding: pure data parallelism — batch B=256 is split 32-per-core across 8
NeuronCores; the small parameter set is replicated.  Training-mode BatchNorm
needs full-batch statistics, so the kernel does two tiny AllReduces (one for
the stage-A convs' stats, one for stage-B).  The first BN (on the input
linear) is handled by replicating that stage's trivially small matmul on
every core, which removes a third AllReduce.

Layout conventions (per core, channels on SBUF partitions):
  - conv activations are stored "t-major": tile column = t*32 + b
  - convT is computed as K shifted matmuls accumulating into one PSUM bank;
    each weight-tap k contributes to output window t in [k, k+L_in); windows
    are emitted as (covered + fresh) split matmuls so per-element PSUM
    has_written semantics stay uniform per instruction.
  - the rank-weighted outer-product head runs per-sample matmuls
    out[(ci,hi), wi] = sum_r (coef*c)[r,ci]*h[r,hi] . w[r,wi]
"""

import os
import threading

import numpy as np

import concourse.bacc as bacc
import concourse.mybir as mybir
import concourse.tile as tile
from concourse.bass_utils import run_bass_kernel_spmd

F32 = mybir.dt.float32
F32R = mybir.dt.float32r
BF16 = mybir.dt.bfloat16

N_CORES = 8
B, NOISE, NCLASS, RANK = 256, 100, 10, 512
R4, R2 = RANK // 4, RANK // 2  # 128, 256
BC = B // N_CORES  # 32 samples per core
EPS = 1e-5

# precision config: "f32" (exact), "f32r" (fp32 storage, fast PE mode on the
# big convs), "bf16" (bf16 weights/activations on the conv + einsum path)
PREC = os.environ.get("KERNEL_PREC", "f32r")
NO_AR = os.environ.get("KERNEL_NO_AR", "0") == "1"
SIM_SAFE = os.environ.get("KERNEL_SIM_SAFE", "0") == "1"

_lock = threading.Lock()
_cache: dict = {}


def _cdt():
    """storage dtype of conv weights/activations on the conv path.
    float32r is fp32 storage that the producers round so the PE can take the
    1-cycle/row fast path (walrus requires the producer-side rounding)."""
    if PREC == "bf16":
        return BF16
    if PREC == "f32r":
        return F32R
    return F32


def _mmbc(ap):
    return ap


# --------------------------------------------------------------------------
# device kernel
# --------------------------------------------------------------------------

def _emit(nc, tc, I, out_ap):
    ADT = _cdt()          # conv activations (h1/h2, latent)
    WDT = _cdt()          # conv weights (host already cast)
    GDT = BF16   # einsum G / w3 path (always bf16: feeds fp32-accum matmuls)
    Alu = mybir.AluOpType
    Act = mybir.ActivationFunctionType

    sb = tc.alloc_tile_pool(name="sb", bufs=1)
    # all PSUM tiles are <= one bank; share 8 rotating bank slots
    psp = tc.alloc_tile_pool(name="ps", bufs=8, space="PSUM")
    dram = tc.alloc_tile_pool(name="dram", bufs=1, space="DRAM")

    def load(name, shape, dtype=F32):
        t = sb.tile(list(shape), dtype, name=f"sb_{name}")
        nc.sync.dma_start(t[:], I[name])
        return t

    # ---- input loads (small/latency-critical first) ----
    noise_my = load("noise_my", (NOISE, BC))
    onehot = load("onehot", (NCLASS, BC))
    lin_wt = load("lin_wt", (NOISE, 128))
    emb_w = load("emb_w", (NCLASS, 128))
    noise_t = load("noise_t", (NOISE, B))
    bn0_g = load("bn0_g", (128, 1))
    bn0_b = load("bn0_b", (128, 1))
    gA = load("gA", (128, 6))
    beA = load("beA", (128, 6))
    coef_p = load("coef_p", (128, 4))
    gB = load("gB", (128, 4))
    beB = load("beB", (128, 4))
    b3h = load("b3h", (128, 4))
    b3w = load("b3w", (128, 4))
    wA_c = load("wA_c", (128, 2 * 3 * 4 * 128), WDT)
    wA_h = load("wA_h", (128, 2 * 16 * 128), WDT)
    wA_w = load("wA_w", (128, 2 * 16 * 128), WDT)
    wB_h = load("wB_h", (128, 16 * 2 * 128), WDT)
    wB_w = load("wB_w", (128, 16 * 2 * 128), WDT)
    wC_h = load("wC_h", (128, 2 * 2 * 4 * 128), WDT)
    wC_w = load("wC_w", (128, 2 * 2 * 4 * 128), WDT)

    eps_c = sb.tile([128, 1], F32, name="eps_c")
    nc.vector.memset(eps_c[:], EPS)

    # ======================= stage 0 (replicated) =========================
    # z_pre^T = lin_w @ noise^T ; full batch for the BN stats, own shard for
    # the actual activations.
    ps_zf = psp.tile([128, B], F32, name="ps_zf", tag="ps")
    nc.tensor.matmul(ps_zf[:], lin_wt[:], noise_t[:], start=True, stop=True)
    ps_zmy = psp.tile([128, BC], F32, name="ps_zmy", tag="ps")
    nc.tensor.matmul(ps_zmy[:], lin_wt[:], noise_my[:], start=True, stop=True)
    ps_lab = psp.tile([128, BC], F32, name="ps_lab", tag="ps")
    nc.tensor.matmul(ps_lab[:], emb_w[:], onehot[:], start=True, stop=True)

    st0 = sb.tile([128, 2], F32, name="st0")
    scr0 = sb.tile([128, 512], F32, name="scr0", tag="scr", bufs=2)
    nc.vector.tensor_scalar(scr0[:, 0:B], ps_zf[:], 1.0, None, Alu.mult, Alu.add,
                            accum_out=st0[:, 0:1])
    nc.scalar.activation(scr0[:, 0:B], ps_zf[:], Act.Square, accum_out=st0[:, 1:2])

    # per-channel BN0 coefficients (count = B; lin_b cancels under BN)
    m0 = sb.tile([128, 2], F32, name="m0")
    nc.vector.tensor_scalar(m0[:], st0[:], 1.0 / B, None, Alu.mult)
    msq0 = sb.tile([128, 1], F32, name="msq0")
    nc.scalar.activation(msq0[:], m0[:, 0:1], Act.Square)
    var0 = sb.tile([128, 1], F32, name="var0")
    nc.vector.tensor_tensor(var0[:], m0[:, 1:2], msq0[:], Alu.subtract)
    std0 = sb.tile([128, 1], F32, name="std0")
    nc.scalar.activation(std0[:], var0[:], Act.Sqrt, bias=eps_c[:])
    rstd0 = sb.tile([128, 1], F32, name="rstd0")
    nc.vector.reciprocal(rstd0[:], std0[:])
    A0 = sb.tile([128, 1], F32, name="A0")
    nc.vector.tensor_tensor(A0[:], rstd0[:], bn0_g[:], Alu.mult)
    t0_ = sb.tile([128, 1], F32, name="t0_")
    nc.vector.tensor_tensor(t0_[:], m0[:, 0:1], A0[:], Alu.mult)
    B0 = sb.tile([128, 1], F32, name="B0")
    nc.vector.tensor_tensor(B0[:], bn0_b[:], t0_[:], Alu.subtract)

    # latent = [lrelu(bn0(z)) ; emb[label]]  (2 chunks of 128 channels)
    lat0 = sb.tile([128, BC], ADT, name="lat0")
    nc.scalar.activation(lat0[:], ps_zmy[:], Act.Identity, bias=B0[:], scale=A0[:])
    nc.vector.scalar_tensor_tensor(lat0[:], lat0[:], 0.01, lat0[:], Alu.mult, Alu.max)
    lat1 = sb.tile([128, BC], ADT, name="lat1")
    nc.vector.tensor_copy(lat1[:], ps_lab[:])
    lat = [lat0, lat1]

    # ======================= stage A convs (L_in=1) =======================
    # x[co, t, b] = sum_ci w[ci, co, t] * latent[ci, b];  psum cols t*32+b
    stA = sb.tile([128, 12], F32, name="stA")

    def stageA_conv(wtile, KA, G, s1cols, name):
        """returns list of evacuated pre-BN sbuf tiles (one per co-group)"""
        outs = []
        for g in range(G):
            ps = psp.tile([128, KA * BC], F32, name=f"psA_{name}{g}", tag="ps")
            n_mm = KA * 2
            i = 0
            for k in range(KA):
                for chunk in range(2):
                    idx = ((chunk * KA + k) * G + g) * 128
                    nc.tensor.matmul(ps[:, k * BC:(k + 1) * BC],
                                     wtile[:, idx:idx + 128], lat[chunk][:],
                                     start=(i == 0), stop=(i == n_mm - 1))
                    i += 1
            xdt = F32 if (ADT == BF16 or G > 1) else ADT
            x = sb.tile([128, KA * BC], xdt, name=f"xA_{name}{g}")
            col = s1cols + g
            nc.vector.tensor_scalar(x[:], ps[:], 1.0, None, Alu.mult, Alu.add,
                                    accum_out=stA[:, col:col + 1])
            scr = sb.tile([128, 512], F32, name=f"scrA_{name}{g}", tag="scr", bufs=2)
            nc.scalar.activation(scr[:, 0:KA * BC], ps[:], Act.Square,
                                 accum_out=stA[:, 6 + col:7 + col])
            outs.append(x)
        return outs

    xA_h = stageA_conv(wA_h, 16, 1, 0, "h")[0]
    xA_w = stageA_conv(wA_w, 16, 1, 1, "w")[0]
    xA_c = stageA_conv(wA_c, 3, 4, 2, "c")

    # ---- AllReduce #1: stage-A BN stats ----
    arA_in = dram.tile([128, 12], F32, name="arA_in")
    arA_out = dram.tile([128, 12], F32, name="arA_out")
    nc.gpsimd.dma_start(arA_in[:], stA[:])
    if NO_AR:
        nc.gpsimd.dma_start(arA_out[:], arA_in[:])
    else:
        nc.gpsimd.collective_compute("AllReduce", Alu.add,
                                     replica_groups=[list(range(N_CORES))],
                                     ins=[arA_in.opt()], outs=[arA_out.opt()])
    stAg = sb.tile([128, 12], F32, name="stAg")
    nc.gpsimd.dma_start(stAg[:], arA_out[:])

    # ---- BN coefficients for stage A (counts: h/w 256*16, c 256*3) ----
    def bn_coeffs(stg, ncols, counts, g_t, be_t, name):
        mean = sb.tile([128, ncols], F32, name=f"mean{name}")
        ex2 = sb.tile([128, ncols], F32, name=f"ex2{name}")
        for sl, cnt in counts:
            nc.vector.tensor_scalar(mean[:, sl], stg[:, sl], 1.0 / cnt, None, Alu.mult)
            s2 = slice(sl.start + ncols, sl.stop + ncols)
            nc.vector.tensor_scalar(ex2[:, sl], stg[:, s2], 1.0 / cnt, None, Alu.mult)
        msq = sb.tile([128, ncols], F32, name=f"msq{name}")
        nc.scalar.activation(msq[:], mean[:], Act.Square)
        var = sb.tile([128, ncols], F32, name=f"var{name}")
        nc.vector.tensor_tensor(var[:], ex2[:], msq[:], Alu.subtract)
        std = sb.tile([128, ncols], F32, name=f"std{name}")
        nc.scalar.activation(std[:], var[:], Act.Sqrt, bias=eps_c[:])
        rstd = sb.tile([128, ncols], F32, name=f"rstd{name}")
        nc.vector.reciprocal(rstd[:], std[:])
        Atl = sb.tile([128, ncols], F32, name=f"A{name}")
        nc.vector.tensor_tensor(Atl[:], rstd[:], g_t[:], Alu.mult)
        tmp = sb.tile([128, ncols], F32, name=f"tmp{name}")
        nc.vector.tensor_tensor(tmp[:], mean[:], Atl[:], Alu.mult)
        Btl = sb.tile([128, ncols], F32, name=f"B{name}")
        nc.vector.tensor_tensor(Btl[:], be_t[:], tmp[:], Alu.subtract)
        return Atl, Btl

    A_A, B_A = bn_coeffs(stAg, 6, [(slice(0, 2), B * 16), (slice(2, 6), B * 3)],
                         gA, beA, "A")
    # fold the rank weights into the c-path coefficients (coef >= 0 commutes
    # with leaky-relu)
    nc.vector.tensor_tensor(A_A[:, 2:6], A_A[:, 2:6], coef_p[:], Alu.mult)
    nc.vector.tensor_tensor(B_A[:, 2:6], B_A[:, 2:6], coef_p[:], Alu.mult)

    def bn_apply(dst, src, Atl, Btl, col, slope):
        nc.scalar.activation(dst, src, Act.Identity,
                             bias=Btl[:, col:col + 1], scale=Atl[:, col:col + 1])
        nc.vector.scalar_tensor_tensor(dst, dst, slope, dst, Alu.mult, Alu.max)

    if ADT != BF16:
        h1, w1 = xA_h, xA_w
        bn_apply(h1[:], h1[:], A_A, B_A, 0, 0.2)
        bn_apply(w1[:], w1[:], A_A, B_A, 1, 0.2)
    else:
        h1 = sb.tile([128, 16 * BC], ADT, name="h1")
        bn_apply(h1[:], xA_h[:], A_A, B_A, 0, 0.2)
        w1 = sb.tile([128, 16 * BC], ADT, name="w1")
        bn_apply(w1[:], xA_w[:], A_A, B_A, 1, 0.2)
    # c~ = coef * lrelu(bn(c_pre)): written b-major (cols b*3+ci) for the head
    cT = []
    for g in range(4):
        ct = sb.tile([128, 3 * BC], F32, name=f"cT{g}")
        src = xA_c[g][:].rearrange("p (c b) -> p c b", b=BC)
        dstv = ct[:].rearrange("p (b c) -> p c b", c=3)
        bn_apply(dstv, src, A_A, B_A, 2 + g, 0.2)
        cT.append(ct)

    # ======================= stage B convs (16 -> 31) =====================
    # y[co, t, b] += sum_ci w2[ci, co, k] * h1[ci, t-k, b]
    stB = sb.tile([128, 16], F32, name="stB")

    def convT(wtile, src_chunks, LI, LO, KK, G, psname, n_bh=2, widx=None):
        """shifted-window convT: returns psum tiles [(g, bh) -> [128, LO*16]]

        Default emits one full-window matmul per (k, chunk) — on HW the
        per-element has_written bits make partially-overlapping windows
        accumulate correctly.  CoreSim asserts uniform pending state per
        matmul, so SIM_SAFE mode splits each k>=1 window into an
        all-covered part plus a single fresh output column.
        """
        pss = {}
        n_ck = len(src_chunks)
        for g in range(G):
            for bh in range(n_bh):
                ps = psp.tile([128, LO * 16], F32, name=f"ps{psname}_{g}_{bh}",
                              tag="ps")
                i = 0
                for k in range(KK):
                    for ck in range(n_ck):
                        lw = wtile[:, widx(ck, k, g):widx(ck, k, g) + 128]
                        src = src_chunks[ck]
                        last = (k == KK - 1 and ck == n_ck - 1)
                        if k == 0 or not SIM_SAFE:
                            rhs = src[:].rearrange("p (t b) -> p t b", b=BC)[
                                :, 0:LI, bh * 16:(bh + 1) * 16]
                            nc.tensor.matmul(ps[:, k * 16:(k + LI) * 16], lw, rhs,
                                             start=(i == 0), stop=last)
                            i += 1
                        else:
                            rhs = src[:].rearrange("p (t b) -> p t b", b=BC)[
                                :, 0:LI - 1, bh * 16:(bh + 1) * 16]
                            nc.tensor.matmul(
                                ps[:, k * 16:(k + LI - 1) * 16], lw, rhs,
                                start=False, stop=False)
                            i += 1
                            rhs2 = src[:, (LI - 1) * BC + bh * 16:
                                       (LI - 1) * BC + bh * 16 + 16]
                            nc.tensor.matmul(
                                ps[:, (k + LI - 1) * 16:(k + LI) * 16], lw, rhs2,
                                start=False, stop=last)
                            i += 1
                pss[(g, bh)] = ps
        return pss

    psB_h = convT(_W(wB_h), [_A(h1)], 16, 31, 16, 2, "Bh",
                  widx=lambda ck, k, g: (k * 2 + g) * 128)
    psB_w = convT(_W(wB_w), [_A(w1)], 16, 31, 16, 2, "Bw",
                  widx=lambda ck, k, g: (k * 2 + g) * 128)

    # evacuate + stats; xB tiles are [128, 31*BC] t-major over full b
    def evacB(pss, G, base, name):
        outs = []
        for g in range(G):
            xdt = F32 if ADT == BF16 else ADT
            x = sb.tile([128, 31 * BC], xdt, name=f"xB_{name}{g}")
            for bh in range(2):
                ps = pss[(g, bh)]
                col = base + g * 2 + bh
                dstv = x[:].rearrange("p (t b) -> p t b", b=BC)[:, :, bh * 16:(bh + 1) * 16]
                nc.vector.tensor_scalar(dstv, ps[:], 1.0, None, Alu.mult, Alu.add,
                                        accum_out=stB[:, col:col + 1])
                scr = sb.tile([128, 512], F32, name=f"scrB_{name}{g}{bh}",
                              tag="scr", bufs=2)
                nc.scalar.activation(scr[:, 0:31 * 16], ps[:], Act.Square,
                                     accum_out=stB[:, 8 + col:9 + col])
            outs.append(x)
        return outs

    xB_h = evacB(psB_h, 2, 0, "h")
    xB_w = evacB(psB_w, 2, 4, "w")

    # ---- AllReduce #2: stage-B BN stats ----
    arB_in = dram.tile([128, 16], F32, name="arB_in")
    arB_out = dram.tile([128, 16], F32, name="arB_out")
    nc.gpsimd.dma_start(arB_in[:], stB[:])
    if NO_AR:
        nc.gpsimd.dma_start(arB_out[:], arB_in[:])
    else:
        nc.gpsimd.collective_compute("AllReduce", Alu.add,
                                     replica_groups=[list(range(N_CORES))],
                                     ins=[arB_in.opt()], outs=[arB_out.opt()])
    stBg = sb.tile([128, 16], F32, name="stBg")
    nc.gpsimd.dma_start(stBg[:], arB_out[:])

    # combine the two batch-half partial sums, then BN coefficients
    stBs = sb.tile([128, 8], F32, name="stBs")
    v = stBg[:].rearrange("p (c two) -> p c two", two=2)
    nc.vector.tensor_tensor(stBs[:], v[:, :, 0:1].squeeze(2), v[:, :, 1:2].squeeze(2),
                            Alu.add)
    A_B, B_B = bn_coeffs(stBs, 4, [(slice(0, 4), B * 31)], gB, beB, "Bst")

    h2, w2 = [], []
    for g in range(2):
        if ADT != BF16:
            t = xB_h[g]
            bn_apply(t[:], t[:], A_B, B_B, g, 0.2)
        else:
            t = sb.tile([128, 31 * BC], ADT, name=f"h2_{g}")
            bn_apply(t[:], xB_h[g][:], A_B, B_B, g, 0.2)
        h2.append(t)
        if ADT != BF16:
            t = xB_w[g]
            bn_apply(t[:], t[:], A_B, B_B, 2 + g, 0.2)
        else:
            t = sb.tile([128, 31 * BC], ADT, name=f"w2_{g}")
            bn_apply(t[:], xB_w[g][:], A_B, B_B, 2 + g, 0.2)
        w2.append(t)

    # ======================= stage C convs (31 -> 32) =====================
    def stageC(wtile, src_chunks, b3t, name, out_dt):
        pss = convT(_W(wtile), [_A(s) for s in src_chunks], 31, 32, 2, 4, name,
                    widx=lambda ck, k, g: ((ck * 2 + k) * 4 + g) * 128)
        outs = []
        for g in range(4):
            h = sb.tile([128, BC * 32], out_dt, name=f"{name}3_{g}")
            for bh in range(2):
                ps = pss[(g, bh)]
                # psum cols t*16+j -> sbuf cols (bh*16+j)*32 + t
                inv = ps[:].rearrange("p (t j) -> p j t", j=16)
                dstv = h[:].rearrange("p (b t) -> p b t", t=32)[
                    :, bh * 16:(bh + 1) * 16, :]
                nc.scalar.activation(dstv, inv, Act.Tanh, bias=b3t[:, g:g + 1])
            outs.append(h)
        return outs

    h3 = stageC(wC_h, h2, b3h, "h", F32)
    GDTt = GDT
    w3 = stageC(wC_w, w2, b3w, "w", GDTt)

    # ======================= rank-weighted outer-product head =============
    # G[q][r, b*96 + ci*32 + hi] = c~[q][r, b*3+ci] * h3[q][r, b*32+hi]
    Gt = []
    for q in range(4):
        g = sb.tile([128, BC * 96], GDTt, name=f"G{q}")
        cv = cT[q][:].rearrange("p (b c) -> p b c", c=3).unsqueeze(3) \
            .broadcast_to([128, BC, 3, 32])
        hv = h3[q][:].rearrange("p (b h) -> p b h", h=32).unsqueeze(2) \
            .broadcast_to([128, BC, 3, 32])
        gv = g[:].rearrange("p (b c h) -> p b c h", c=3, h=32)
        nc.vector.tensor_tensor(gv, cv, hv, Alu.mult)
        Gt.append(g)

    outsb = sb.tile([96, BC * 32], F32, name="outsb")
    for grp in range(BC // 4):
        po = psp.tile([96, 4 * 32], F32, name=f"po{grp}", tag="ps")
        for j in range(4):
            bb = grp * 4 + j
            for q in range(4):
                nc.tensor.matmul(po[:, j * 32:(j + 1) * 32],
                                 Gt[q][:, bb * 96:(bb + 1) * 96],
                                 w3[q][:, bb * 32:(bb + 1) * 32],
                                 start=(j == 0 and q == 0),
                                 stop=(j == 3 and q == 3))
        nc.vector.tensor_copy(outsb[:, grp * 128:(grp + 1) * 128], po[:])

    # out[b, (ci,hi), wi] <- outsb[(ci,hi), b*32+wi]
    out_v = out_ap.rearrange("b c h w -> (c h) b w")
    nc.sync.dma_start(out_v, outsb[:].rearrange("p (b w) -> p b w", w=32))

    sb.release()
    psp.release()
    dram.release()


def _W(wtile):
    return _WView(wtile)


class _WView:
    """weight tile wrapper applying the f32r bitcast at slice time"""

    def __init__(self, t):
        self.t = t

    def __getitem__(self, key):
        return _mmbc(self.t[key])


class _AView:
    """activation tile wrapper applying the f32r bitcast at slice time"""

    def __init__(self, t):
        self.t = t

    def __getitem__(self, key):
        return _mmbc(self.t[key])


def _A(t):
    return _AView(t)


# --------------------------------------------------------------------------
# host side
# --------------------------------------------------------------------------

def _build_module():
    nc = bacc.Bacc("TRN2", target_bir_lowering=False, debug=False,
                   num_devices=N_CORES)
    WDT = _cdt()
    specs = {
        "noise_my": ((NOISE, BC), F32), "onehot": ((NCLASS, BC), F32),
        "lin_wt": ((NOISE, 128), F32), "emb_w": ((NCLASS, 128), F32),
        "noise_t": ((NOISE, B), F32),
        "bn0_g": ((128, 1), F32), "bn0_b": ((128, 1), F32),
        "gA": ((128, 6), F32), "beA": ((128, 6), F32),
        "coef_p": ((128, 4), F32), "gB": ((128, 4), F32), "beB": ((128, 4), F32),
        "b3h": ((128, 4), F32), "b3w": ((128, 4), F32),
        "wA_c": ((128, 3072), WDT), "wA_h": ((128, 4096), WDT),
        "wA_w": ((128, 4096), WDT),
        "wB_h": ((128, 4096), WDT), "wB_w": ((128, 4096), WDT),
        "wC_h": ((128, 2048), WDT), "wC_w": ((128, 2048), WDT),
    }
    I = {}
    for name, (shape, dt) in specs.items():
        I[name] = nc.dram_tensor(name, list(shape), dt, kind="ExternalInput").ap()
    out = nc.dram_tensor("out", [BC, 3, 32, 32], F32, kind="ExternalOutput")
    with tile.TileContext(nc) as tc:
        _emit(nc, tc, I, out.ap())
    nc.compile()
    return nc


def _np(x):
    return np.ascontiguousarray(np.asarray(x, dtype=np.float32))


def _pack_inputs(inputs):
    """host-side layout packing -> (replicated dict, per-core dicts)"""
    wnp = np.dtype(mybir.dt.np(_cdt()))
    noise = _np(inputs["noise"])
    label = np.asarray(inputs["label"]).astype(np.int64)

    c_w1 = _np(inputs["c_w1"])   # (256, 512, 3)
    h_w1 = _np(inputs["h_w1"])   # (256, 128, 16)
    w_w1 = _np(inputs["w_w1"])
    h_w2 = _np(inputs["h_w2"])   # (128, 256, 16)
    w_w2 = _np(inputs["w_w2"])
    h_w3 = _np(inputs["h_w3"])   # (256, 512, 2)
    w_w3 = _np(inputs["w_w3"])

    def packA_c(w):   # -> [ci_in, (chunk, k, g, co_in)]
        return np.ascontiguousarray(
            w.reshape(2, 128, 4, 128, 3).transpose(1, 0, 4, 2, 3).reshape(128, -1))

    def packA_h(w):   # (256,128,16) -> [ci_in, (chunk, k, co)]
        return np.ascontiguousarray(
            w.reshape(2, 128, 128, 16).transpose(1, 0, 3, 2).reshape(128, -1))

    def packB(w):     # (128,256,16) -> [ci, (k, g, co_in)]
        return np.ascontiguousarray(
            w.reshape(128, 2, 128, 16).transpose(0, 3, 1, 2).reshape(128, -1))

    def packC(w):     # (256,512,2) -> [ci_in, (chunk, k, g, co_in)]
        return np.ascontiguousarray(
            w.reshape(2, 128, 4, 128, 2).transpose(1, 0, 4, 2, 3).reshape(128, -1))

    def col128(*arrs):
        return np.ascontiguousarray(
            np.concatenate([a.reshape(-1, 128).T for a in arrs], axis=1))

    rep = {
        "lin_wt": _np(inputs["lin_w"]).T.copy(),
        "emb_w": _np(inputs["emb"]),
        "noise_t": noise.T.copy(),
        "bn0_g": _np(inputs["bn0_g"]).reshape(128, 1),
        "bn0_b": _np(inputs["bn0_b"]).reshape(128, 1),
        "gA": col128(_np(inputs["h_g1"]), _np(inputs["w_g1"]), _np(inputs["c_g1"])),
        "beA": col128(_np(inputs["h_be1"]), _np(inputs["w_be1"]), _np(inputs["c_be1"])),
        "coef_p": col128(_np(inputs["coef"])),
        "gB": col128(_np(inputs["h_g2"]), _np(inputs["w_g2"])),
        "beB": col128(_np(inputs["h_be2"]), _np(inputs["w_be2"])),
        "b3h": col128(_np(inputs["h_b3"])),
        "b3w": col128(_np(inputs["w_b3"])),
        "wA_c": packA_c(c_w1).astype(wnp),
        "wA_h": packA_h(h_w1).astype(wnp),
        "wA_w": packA_h(w_w1).astype(wnp),
        "wB_h": packB(h_w2).astype(wnp),
        "wB_w": packB(w_w2).astype(wnp),
        "wC_h": packC(h_w3).astype(wnp),
        "wC_w": packC(w_w3).astype(wnp),
    }

    noise_t = rep["noise_t"]
    per_core = []
    for c in range(N_CORES):
        sl = slice(c * BC, (c + 1) * BC)
        oh = (label[sl][None, :] == np.arange(NCLASS)[:, None]).astype(np.float32)
        per_core.append({
            "noise_my": np.ascontiguousarray(noise_t[:, sl]),
            "onehot": np.ascontiguousarray(oh),
            **rep,
        })
    return per_core


def kernel(**inputs) -> np.ndarray:
    with _lock:
        key = (PREC, NO_AR, SIM_SAFE)
        nc = _cache.get(key)
        if nc is None:
            nc = _build_module()
            _cache[key] = nc
    in_maps = _pack_inputs(inputs)
    res = run_bass_kernel_spmd(nc, in_maps, core_ids=list(range(N_CORES)))
    return np.concatenate([r["out"] for r in res.results], axis=0)
